# revision 1
# baseline (speedup 1.0000x reference)
"""Trainium2 Bass kernel: out = e + e @ B @ A^T  (low-rank residual update).

e: [4, 4096, 4096] f32, A/B: [4096, 16] f32.
Data-parallel over rows (4*4096=16384 rows -> 2048 rows/core on 8 cores).

Per core, supertiles of 256 rows (2 row-blocks), software-pipelined:
  stage1(st): one 4MB HWDGE load (SP ring); per macro-chunk of 512 d-cols:
    8 PE transposes (fp32) -> PSUM, one ACT copy PSUM->SBUF (casting to
    f32r), 4 rank-accumulating matmuls  t^T[16,256] += B_k^T @ e^T_k.
  stage2(st): t slices @ A^T chunks -> PSUM (f32r), DVE adds e_seg + y ->
    separate out tile, per-row-block 2MB HWDGE stores (ACT ring — measured
    22 us/pass faster than SWDGE/Pool stores).
  Emission interleaves stage2(st-1) pairs into stage1(st)'s macro loop so
  every in-order engine stream alternates ready work; stage2(0) is emitted
  eagerly after stage1(1)'s load to shorten the pipeline fill.
Const B/A^T loads go first on the SP ring (the shared DMA engines grant
FIFO by ready-time; a queued store never head-of-line blocks a load), as
f32, then converted once on-chip to f32r (walrus requires f32r matmult
operands to be produced as f32r).

Modeled (TimelineSim): 200 us/core vs 189 us pure-DMA floor (96% DMA busy).
Measured on HW (in-NEFF repeat-loop slope): 202.5 us/pass, vs ~214 us for a
DMA-only kernel with the same transfer sizes — compute fully hidden.
"""

import sys

sys.path.insert(0, "/opt/trn_rl_repo")

import numpy as np

import concourse.bass as bass
import concourse.mybir as mybir
import concourse.tile as tile
from concourse.masks import make_identity


def _split_waits(nc, max_w=1):
    """The walrus in this container rejects instructions carrying more than
    ~2 sync-waits. Hoist extra waits onto same-engine NOPs placed directly
    before the offending instruction (engines execute their stream in
    order, so this is semantics-preserving)."""
    for f in nc.m.functions:
        for blk in f.blocks:
            insts = blk.instructions
            out = []
            changed = False
            for inst in insts:
                si = inst.sync_info
                if si is not None and si.on_wait and len(si.on_wait) > max_w:
                    waits = list(si.on_wait)
                    for j, w in enumerate(waits[max_w:]):
                        out.append(
                            mybir.InstNoOp(
                                name=f"{inst.name}-wsplit{j}",
                                sync_info=mybir.SyncInfo(on_wait=[w], on_update=[]),
                                bass_nofuse=True,
                                engine=inst.engine,
                            )
                        )
                    si.on_wait = waits[:max_w]
                    changed = True
                out.append(inst)
            if changed:
                blk.instructions = out


DIM = 4096
RANK = 16
N_CORES = 8
ROWS_TOTAL = 4 * 4096
ROWS_PER_CORE = ROWS_TOTAL // N_CORES  # 2048

MM_DT = mybir.dt.float32r
F32 = mybir.dt.float32


def build_nc(rows_per_core=ROWS_PER_CORE, st_rows=256, mm_dt=MM_DT, split_waits=True,
             macro=4, e_bufs=3, o_bufs=4, ets_bufs=2, y_bufs=2, interleave=True,
             reps=1, trans_f32r=False, load_span=1, store_eng="scalar"):
    # trans_f32r: run the PE transposes in float32r (1.5 cycles/row vs 2.0).
    # The BIR verifier demands f32r-matmult operands be *produced* as f32r,
    # so e_in/et/ident/trp are all declared f32r (same bits as f32); non-
    # matmult readers of et bitcast back to f32.
    E_DT = mybir.dt.float32r if trans_f32r else F32
    assert st_rows % 128 == 0 and rows_per_core % st_rows == 0
    rb = st_rows // 128
    n_st = rows_per_core // st_rows
    kc = DIM // 128
    assert kc % macro == 0
    n_macro = kc // macro
    nch = DIM // 512
    n_pairs = rb * nch           # stage2 (b, n) pairs per supertile
    inplace = o_bufs == 0        # adds write et; whole-supertile stores
    assert (n_st * reps) % load_span == 0

    nc = bass.Bass("TRN2", target_bir_lowering=False, debug=False)
    e_in = nc.dram_tensor("e_in", [rows_per_core, DIM], E_DT, kind="ExternalInput")
    b_in = nc.dram_tensor("b_in", [DIM, RANK], F32, kind="ExternalInput")
    at_in = nc.dram_tensor("at_in", [RANK, DIM], F32, kind="ExternalInput")
    out_d = nc.dram_tensor("out_d", [rows_per_core, DIM], F32, kind="ExternalOutput")

    with tile.TileContext(nc) as tc:
        with (
            tc.tile_pool(name="const", bufs=1) as cpool,
            tc.tile_pool(name="epool", bufs=e_bufs) as epool,
            tc.tile_pool(name="opool", bufs=max(o_bufs, 1)) as opool,
            tc.tile_pool(name="etpool", bufs=ets_bufs) as etpool,
            tc.tile_pool(name="ttpool", bufs=2) as ttpool,
            tc.tile_pool(name="pstr", bufs=2, space="PSUM") as pstr,
            tc.tile_pool(name="pst", bufs=2, space="PSUM") as pst,
            tc.tile_pool(name="psy", bufs=y_bufs, space="PSUM") as psy,
        ):
            ident = cpool.tile([128, 128], E_DT, name="ident")
            make_identity(nc, ident)

            # Const loads go on the SP HWDGE ring *before* the first e load:
            # the shared DMA engines grant FIFO by ready-time, and the first
            # stage-1 matmul needs b_sb. HWDGE can't cast and walrus requires
            # FP32r matmult operands to be *produced* as f32r, so load f32 and
            # convert once on-chip (engines are idle during the fill anyway).
            # The f32 staging tiles live in a scoped pool so their SBUF is
            # returned before the big e tiles are allocated.
            b_sb = cpool.tile([128, kc * RANK], mm_dt, name="b_sb")
            at_sb = cpool.tile([RANK, DIM], mm_dt, name="at_sb")
            with tc.tile_pool(name="cstage", bufs=1) as spool:
                b_f32 = spool.tile([128, kc * RANK], F32, name="b_f32")
                nc.sync.dma_start(
                    out=b_f32.rearrange("p (k j) -> p k j", j=RANK),
                    in_=b_in.ap().rearrange("(k p) j -> p k j", p=128),
                )
                at_f32 = spool.tile([RANK, DIM], F32, name="at_f32")
                nc.sync.dma_start(out=at_f32, in_=at_in.ap()[:, :])
                nc.vector.tensor_copy(out=b_sb, in_=b_f32)
                nc.scalar.copy(at_sb, at_f32)

            e_ap = e_in.ap()
            o_ap = out_d.ap()

            ctx = {}  # st -> dict(et, tps, tts, ot map)

            def emit_load(st):
                # load load_span supertiles' rows in one DMA; later
                # supertiles in the span get column-offset views of the tile
                if st % load_span == 0:
                    r0 = (st % n_st) * st_rows
                    et = epool.tile([128, load_span * rb * DIM], E_DT, name="et")
                    nc.sync.dma_start(
                        out=et.rearrange("p (b c) -> p b c", c=DIM),
                        in_=e_ap[r0 : r0 + load_span * st_rows, :].rearrange(
                            "(b p) c -> p b c", p=128
                        ),
                    )
                    ctx["_span_et"] = et
                et = ctx["_span_et"][
                    :, (st % load_span) * rb * DIM : (st % load_span + 1) * rb * DIM
                ]
                tps = pst.tile([RANK, st_rows], F32, name="tps")
                ctx[st] = {"et": et, "tps": tps, "ot": {}}

            def emit_s1_macro(st, m):
                et, tps = ctx[st]["et"], ctx[st]["tps"]
                trp = pstr.tile([128, macro * st_rows], E_DT, name="trp")
                for ks in range(macro):
                    k = macro * m + ks
                    for b in range(rb):
                        nc.tensor.transpose(
                            trp[:, ks * st_rows + b * 128 : ks * st_rows + (b + 1) * 128],
                            et[:, b * DIM + k * 128 : b * DIM + (k + 1) * 128],
                            ident,
                        )
                ets = etpool.tile([128, macro * st_rows], mm_dt, name="ets")
                nc.scalar.copy(ets, trp)
                for ks in range(macro):
                    k = macro * m + ks
                    nc.tensor.matmul(
                        tps,
                        b_sb[:, k * RANK : (k + 1) * RANK],
                        ets[:, ks * st_rows : (ks + 1) * st_rows],
                        start=(k == 0),
                        stop=(k == kc - 1),
                    )

            def emit_s2_head(st):
                tts = ttpool.tile([RANK, st_rows], mm_dt, name="tts")
                nc.vector.tensor_copy(out=tts, in_=ctx[st]["tps"])
                ctx[st]["tts"] = tts

            def emit_s2_pair(st, p):
                b, n = divmod(p, nch)
                c = ctx[st]
                yp = psy.tile([128, 512], F32, name="yp")
                nc.tensor.matmul(
                    yp,
                    c["tts"][:, b * 128 : (b + 1) * 128],
                    at_sb[:, n * 512 : (n + 1) * 512],
                    start=True,
                    stop=True,
                )
                sl_e = slice(b * DIM + n * 512, b * DIM + (n + 1) * 512)
                e_seg = c["et"][:, sl_e]
                if trans_f32r:
                    e_seg = e_seg.bitcast(F32)
                if inplace:
                    nc.vector.tensor_add(out=e_seg, in0=e_seg, in1=yp)
                    # store in row-block-pair halves so the et WAR releases
                    # as soon as each half's adds are done
                    if (p + 1) % (2 * nch) == 0:
                        h = (p + 1) // (2 * nch) - 1     # half index
                        r0 = (st % n_st) * st_rows + h * 256
                        et_store = c["et"][:, h * 2 * DIM : (h + 1) * 2 * DIM]
                        if trans_f32r:
                            et_store = et_store.bitcast(F32)
                        nc.gpsimd.dma_start(
                            out=o_ap[r0 : r0 + 256, :].rearrange(
                                "(b p) c -> p b c", p=128
                            ),
                            in_=et_store.rearrange("p (b c) -> p b c", c=DIM),
                        )
                        if p == n_pairs - 1:
                            del ctx[st]
                    return
                if n == 0:
                    c["ot"][b] = opool.tile([128, DIM], F32, name="ot")
                ot = c["ot"][b]
                sl_o = slice(n * 512, (n + 1) * 512)
                nc.vector.tensor_add(out=ot[:, sl_o], in0=e_seg, in1=yp)
                if n == nch - 1:
                    r0 = (st % n_st) * st_rows + b * 128
                    getattr(nc, store_eng).dma_start(out=o_ap[r0 : r0 + 128, :], in_=ot)
                    if b == rb - 1:
                        del ctx[st]

            total_st = n_st * reps  # reps>1: timing-only in-NEFF repeat loop
            if interleave:
                for st in range(total_st):
                    emit_load(st)
                    if 1 <= st <= load_span:
                        # Eager stage2(st-1) during the fill: the steady-state
                        # interleave would stretch the first supertiles' adds
                        # across a whole stage1, delaying the first et-slot
                        # release (and first store) while the DMA sits idle.
                        emit_s2_head(st - 1)
                        for p in range(n_pairs):
                            emit_s2_pair(st - 1, p)
                    for m in range(n_macro):
                        emit_s1_macro(st, m)
                        if st > load_span:
                            if m == 0:
                                emit_s2_head(st - 1)
                            for p in range(m * n_pairs // n_macro,
                                           (m + 1) * n_pairs // n_macro):
                                emit_s2_pair(st - 1, p)
                emit_s2_head(total_st - 1)
                for p in range(n_pairs):
                    emit_s2_pair(total_st - 1, p)
            else:
                for st in range(total_st):
                    emit_load(st)
                    for m in range(n_macro):
                        emit_s1_macro(st, m)
                    if st > 0:
                        emit_s2_head(st - 1)
                        for p in range(n_pairs):
                            emit_s2_pair(st - 1, p)
                emit_s2_head(total_st - 1)
                for p in range(n_pairs):
                    emit_s2_pair(total_st - 1, p)

    if split_waits:
        _split_waits(nc)
    return nc


_NC_CACHE = {}


def _get_nc(rows_per_core=ROWS_PER_CORE, st_rows=256, mm_dt=MM_DT):
    key = (rows_per_core, st_rows, mm_dt)
    if key not in _NC_CACHE:
        _NC_CACHE[key] = build_nc(rows_per_core, st_rows, mm_dt)
    return _NC_CACHE[key]


def kernel(e, A, B):
    from concourse.bass_utils import run_bass_kernel_spmd

    e = np.asarray(e, dtype=np.float32)
    A = np.asarray(A, dtype=np.float32)
    B = np.asarray(B, dtype=np.float32)
    batch, seq, dim = e.shape
    rows = batch * seq
    e2 = np.ascontiguousarray(e.reshape(rows, dim))
    at = np.ascontiguousarray(A.T)

    rpc = rows // N_CORES
    in_maps = [
        {
            "e_in": np.ascontiguousarray(e2[i * rpc : (i + 1) * rpc]),
            "b_in": B,
            "at_in": at,
        }
        for i in range(N_CORES)
    ]
    nc = _get_nc(rpc)
    res = run_bass_kernel_spmd(nc, in_maps, core_ids=list(range(N_CORES)))
    out = np.concatenate([res.results[i]["out_d"] for i in range(N_CORES)], axis=0)
    return out.reshape(batch, seq, dim).astype(np.float32)

